# revision 9
# baseline (speedup 1.0000x reference)
"""Trainium2 Bass kernel for nn_AttentionResidual (sparse_attention).

Computes, for V:(n=8,b=4,s=2048,d=1024), proj:(12,1024), scale:(1024,), block_idx:
    w       = proj[min(block_idx, 11)]
    rms     = sqrt(mean(V^2, axis=-1) + 1e-5)
    logits  = sum_d (w*scale)[d] * V[...,d] / rms
    weights = softmax(logits, axis=n)
    out     = sum_n weights[n] * V[n]                       # (b,s,d)

Sharding: data-parallel over the 8192 (b,s) positions across 8 NeuronCores
(1024 positions per core). V is cast to fp16 on the host (rel-err ~9e-3,
under the 2e-2 gate) halving HBM traffic and enabling fp16 PE matmuls.

Per core: 8 position blocks of 128. DVE does the ws-dot (STT+accum, 1x),
ACT the sum-of-squares (Square+accum). Softmax small-ops batch per quad
(group sizes [3,3,1,1] - the small trailing groups shrink the pipeline
tail). diag(e_n) stationary tiles are built in one broadcast-AP TT per
group; the weighted sum runs as 16 accumulating fp16 matmuls per block;
1/sum(e) is folded into the PSUM->SBUF copy (split ACT/DVE). All of V
(16MB fp16) stays resident in SBUF, so every DMA can issue upfront.
"""

import numpy as np

N, B, S, D = 8, 4, 2048, 1024
NCORES = 8
BS = B * S            # 8192 flattened (b,s) positions
PER = BS // NCORES    # 1024 positions per core
PB = PER // 128       # 8 position blocks per core
QUADS = [(0, 3), (3, 6), (6, 7), (7, 8)]   # block groups for softmax batching
EPS = 1e-5

# blocks whose PSUM->SBUF copy runs on DVE (rest on ACT) to balance engines
DVE_COPY = {3}
# blocks whose n==0 sum-of-squares runs on DVE (STT) instead of ACT
DVE_SQ = {0, 1, 2, 4, 5, 6, 7}

_cache = {}


def _build():
    import concourse.tile as tile
    from concourse import bacc, mybir

    OP = mybir.AluOpType
    A = mybir.ActivationFunctionType
    X = mybir.AxisListType.X
    f32 = mybir.dt.float32
    f16 = mybir.dt.float16

    from concourse.hw_specs import get_activation_tables

    nc = bacc.Bacc(
        "TRN2",
        target_bir_lowering=False,
        debug=False,
        enable_asserts=False,
        num_devices=NCORES,
    )
    v = nc.dram_tensor("v", [N, PER, D], f16, kind="ExternalInput").ap()
    wsb = nc.dram_tensor("wsb", [128, D], f16, kind="ExternalInput").ap()
    ident = nc.dram_tensor("ident", [128, 128], f16, kind="ExternalInput").ap()
    o = nc.dram_tensor("o", [PER, D], f16, kind="ExternalOutput").ap()

    act_set_id = list(get_activation_tables(nc.m.arch).keys()).index(
        "natural_log_exp_and_others"
    )

    with tile.TileContext(nc) as tc:
        with (
            tc.tile_pool(name="vp", bufs=PB) as vp,
            tc.tile_pool(name="wp", bufs=1) as wp,
            tc.tile_pool(name="dsc", bufs=1) as dsc,
            tc.tile_pool(name="sqc", bufs=1) as sqc,
            tc.tile_pool(name="dgp", bufs=2) as dgp,
            tc.tile_pool(name="op_", bufs=3) as outp,
            tc.tile_pool(name="stq", bufs=2) as stq,
            tc.tile_pool(name="ps", bufs=3, space="PSUM") as ps,
        ):
            nc.scalar.add_instruction(
                mybir.InstLoadActFuncSet(
                    name=nc.get_next_instruction_name(),
                    ins=[],
                    outs=[],
                    act_func_set_id=act_set_id,
                )
            )
            # Block 0 streams in per-n (first stats start after 256KB, not
            # 2MB); wsb rides between the first two chunks.
            vts = {}
            t0 = vp.tile([128, N, D], f16, tag="v", name="v_0")
            vts[0] = t0
            nc.sync.dma_start(t0[:, 0, :], v[0, 0:128, :])
            wt = wp.tile([128, D], f16, tag="w")
            nc.sync.dma_start(wt[:], wsb[:])
            for n in range(1, N):
                nc.sync.dma_start(t0[:, n, :], v[n, 0:128, :])
            idt = wp.tile([128, 128], f16, tag="id")
            nc.sync.dma_start(idt[:], ident[:])
            epsb = wp.tile([128, 1], f32, tag="eps")
            nc.vector.memset(epsb[:], EPS)
            for pp in range(1, PB):
                t = vp.tile([128, N, D], f16, tag="v", name=f"v_{pp}")
                src = v[:, pp * 128:(pp + 1) * 128, :].rearrange("n p d -> p n d")
                nc.sync.dma_start(t[:], src)
                vts[pp] = t

            stats = {}

            def emit_block_stats(qi, pp):
                lo, hi = QUADS[qi]
                W = (hi - lo) * N
                if qi not in stats:
                    stats[qi] = (
                        stq.tile([128, W], f32, tag=f"ssq{qi}", name=f"ssq_{qi}"),
                        stq.tile([128, W], f32, tag=f"dotq{qi}", name=f"dotq_{qi}"),
                    )
                ssq, dotq = stats[qi]
                if True:
                    j = (pp - lo) * N
                    for n in range(N):
                        dst = dsc.tile([128, D], f16, tag="ds")
                        nc.vector.scalar_tensor_tensor(
                            out=dst[:], in0=vts[pp][:, n, :], scalar=1.0,
                            in1=wt[:], op0=OP.mult, op1=OP.mult,
                            accum_out=dotq[:, j + n:j + n + 1],
                        )
                        if n == 0 and pp in DVE_SQ:
                            dsq = dsc.tile([128, D], f16, tag="ds2")
                            nc.vector.scalar_tensor_tensor(
                                out=dsq[:], in0=vts[pp][:, n, :], scalar=1.0,
                                in1=vts[pp][:, n, :], op0=OP.mult, op1=OP.mult,
                                accum_out=ssq[:, j + n:j + n + 1],
                            )
                        else:
                            sqt = sqc.tile([128, D], f16, tag="sq")
                            nc.scalar.activation(
                                sqt[:], vts[pp][:, n, :], A.Square,
                                accum_out=ssq[:, j + n:j + n + 1],
                            )

            def emit_tail(qi, ssq, dotq):
                lo, hi = QUADS[qi]
                nq = hi - lo
                W = nq * N
                lnq = stq.tile([128, W], f32, tag=f"lnq{qi}", name=f"lnq_{qi}")
                nc.scalar.activation(
                    lnq[:], ssq[:], A.Ln, bias=epsb[:], scale=1.0 / D
                )
                y0q = stq.tile([128, W], f32, tag=f"y0q{qi}", name=f"y0q_{qi}")
                nc.scalar.activation(y0q[:], lnq[:], A.Exp, scale=-0.5)
                lgq = stq.tile([128, W], f32, tag=f"lgq{qi}", name=f"lgq_{qi}")
                nc.vector.tensor_mul(lgq[:], dotq[:], y0q[:])
                nmq = stq.tile([128, nq], f32, tag=f"nmq{qi}", name=f"nmq_{qi}")
                nc.vector.tensor_reduce(
                    nmq[:], lgq[:].rearrange("p (b n) -> p b n", b=nq),
                    X, OP.max, negate=True,
                )
                lgs = stq.tile([128, W], f32, tag=f"lgs{qi}", name=f"lgs_{qi}")
                nm_b = nmq[:].unsqueeze(2).broadcast_to([128, nq, N])
                nc.vector.tensor_tensor(
                    lgs[:].rearrange("p (b n) -> p b n", b=nq),
                    lgq[:].rearrange("p (b n) -> p b n", b=nq),
                    nm_b, OP.add,
                )
                eq = stq.tile([128, W], f16, tag=f"eq{qi}", name=f"eq_{qi}")
                nc.scalar.activation(eq[:], lgs[:], A.Exp)
                smq = stq.tile([128, nq], f32, tag=f"smq{qi}", name=f"smq_{qi}")
                nc.vector.tensor_reduce(
                    smq[:], eq[:].rearrange("p (b n) -> p b n", b=nq),
                    X, OP.add,
                )
                rsq = stq.tile([128, nq], f32, tag=f"rsq{qi}", name=f"rsq_{qi}")
                nc.vector.reciprocal(rsq[:], smq[:])

                dgq = dgp.tile(
                    [128, W * 128], f16, tag="dg", name=f"dg_{qi}"
                )
                e_b = eq[:].unsqueeze(2).broadcast_to([128, W, 128])
                i_b = idt[:].unsqueeze(1).broadcast_to([128, W, 128])
                nc.gpsimd.tensor_tensor(
                    dgq[:].rearrange("p (w c) -> p w c", w=W), e_b, i_b, OP.mult
                )

                for pp in range(lo, hi):
                    b = pp - lo
                    acc_ps = ps.tile([128, D], f32, tag="acc")
                    for n in range(N):
                        dgsl = dgq[:, (b * N + n) * 128:(b * N + n + 1) * 128]
                        nc.tensor.matmul(
                            acc_ps[:, 0:512], dgsl, vts[pp][:, n, 0:512],
                            start=(n == 0), stop=(n == N - 1),
                        )
                        nc.tensor.matmul(
                            acc_ps[:, 512:1024], dgsl, vts[pp][:, n, 512:1024],
                            start=(n == 0), stop=(n == N - 1),
                        )
                    outt = outp.tile([128, D], f16, tag="o")
                    rs_b = rsq[:, b:b + 1]
                    if pp in DVE_COPY:
                        nc.vector.tensor_scalar(
                            outt[:], acc_ps[:], rs_b, None, OP.mult
                        )
                    else:
                        nc.scalar.activation(
                            outt[:], acc_ps[:], A.Copy, scale=rs_b
                        )
                    nc.sync.dma_start(o[pp * 128:(pp + 1) * 128, :], outt[:])

            # one-block-lookahead emission: quad qi's tail is emitted after
            # the first block of quad qi+1's stats, so each engine has ~one
            # block of queued stat work while the cross-engine softmax chain
            # of the previous quad resolves.
            stats = {}
            pending = None
            for qi, (lo, hi) in enumerate(QUADS):
                for pp in range(lo, hi):
                    emit_block_stats(qi, pp)
                    if pp == lo and pending is not None:
                        emit_tail(pending, *stats[pending])
                        pending = None
                pending = qi
            emit_tail(pending, *stats[pending])

    nc.compile()
    return nc


def get_program():
    if "nc" not in _cache:
        _cache["nc"] = _build()
    return _cache["nc"]


def make_in_maps(V, proj, scale, block_idx):
    V = np.asarray(V)
    proj = np.asarray(proj, dtype=np.float32)
    scale = np.asarray(scale, dtype=np.float32)
    idx = min(int(block_idx), proj.shape[0] - 1)
    ws = (proj[idx] * scale).astype(np.float16)
    wsb = np.ascontiguousarray(np.broadcast_to(ws, (128, D)))
    eye = np.eye(128, dtype=np.float16)
    Vf = V.reshape(N, BS, D)
    return [
        {
            "v": np.ascontiguousarray(Vf[:, k * PER:(k + 1) * PER, :]).astype(
                np.float16
            ),
            "wsb": wsb,
            "ident": eye,
        }
        for k in range(NCORES)
    ]


def kernel(V, proj, scale, block_idx):
    from concourse.bass_utils import run_bass_kernel_spmd

    nc = get_program()
    in_maps = make_in_maps(V, proj, scale, block_idx)
    res = run_bass_kernel_spmd(nc, in_maps, core_ids=list(range(NCORES)))
    _cache["last_exec_time_ns"] = res.exec_time_ns
    _cache["last_results"] = res
    out = np.concatenate([res.results[k]["o"] for k in range(NCORES)], axis=0)
    return out.reshape(B, S, D).astype(np.float32)


# revision 10
# speedup vs baseline: 1.0383x; 1.0383x over previous
"""Trainium2 Bass kernel for nn_AttentionResidual (sparse_attention).

Computes, for V:(n=8,b=4,s=2048,d=1024), proj:(12,1024), scale:(1024,), block_idx:
    w       = proj[min(block_idx, 11)]
    rms     = sqrt(mean(V^2, axis=-1) + 1e-5)
    logits  = sum_d (w*scale)[d] * V[...,d] / rms
    weights = softmax(logits, axis=n)
    out     = sum_n weights[n] * V[n]                       # (b,s,d)

Sharding: data-parallel over the 8192 (b,s) positions across 8 NeuronCores
(1024 positions per core). V is cast to fp16 on the host (rel-err ~9e-3,
under the 2e-2 gate) halving HBM traffic and enabling fp16 PE matmuls.

Per core: 8 position blocks of 128. DVE does the ws-dot (STT+accum, 1x),
ACT the sum-of-squares (Square+accum). Softmax small-ops batch per quad
(group sizes [3,3,1,1] - the small trailing groups shrink the pipeline
tail). diag(e_n) stationary tiles are built in one broadcast-AP TT per
group; the weighted sum runs as 16 accumulating fp16 matmuls per block;
1/sum(e) is folded into the PSUM->SBUF copy (split ACT/DVE). All of V
(16MB fp16) stays resident in SBUF, so every DMA can issue upfront.
"""

import numpy as np

N, B, S, D = 8, 4, 2048, 1024
NCORES = 8
BS = B * S            # 8192 flattened (b,s) positions
PER = BS // NCORES    # 1024 positions per core
PB = PER // 128       # 8 position blocks per core
QUADS = [(0, 3), (3, 6), (6, 7), (7, 8)]   # block groups for softmax batching
EPS = 1e-5

# blocks whose PSUM->SBUF copy runs on DVE (rest on ACT) to balance engines
DVE_COPY = {3}
# blocks whose n==0 sum-of-squares runs on DVE (STT) instead of ACT
DVE_SQ = {0, 2, 4, 6}

_cache = {}


def _build():
    import concourse.tile as tile
    from concourse import bacc, mybir

    OP = mybir.AluOpType
    A = mybir.ActivationFunctionType
    X = mybir.AxisListType.X
    f32 = mybir.dt.float32
    f16 = mybir.dt.float16

    from concourse.hw_specs import get_activation_tables

    nc = bacc.Bacc(
        "TRN2",
        target_bir_lowering=False,
        debug=False,
        enable_asserts=False,
        num_devices=NCORES,
    )
    v = nc.dram_tensor("v", [N, PER, D], f16, kind="ExternalInput").ap()
    wsb = nc.dram_tensor("wsb", [128, D], f16, kind="ExternalInput").ap()
    ident = nc.dram_tensor("ident", [128, 128], f16, kind="ExternalInput").ap()
    o = nc.dram_tensor("o", [PER, D], f16, kind="ExternalOutput").ap()

    act_set_id = list(get_activation_tables(nc.m.arch).keys()).index(
        "natural_log_exp_and_others"
    )

    with tile.TileContext(nc) as tc:
        with (
            tc.tile_pool(name="vp", bufs=PB) as vp,
            tc.tile_pool(name="wp", bufs=1) as wp,
            tc.tile_pool(name="dsc", bufs=1) as dsc,
            tc.tile_pool(name="sqc", bufs=1) as sqc,
            tc.tile_pool(name="dgp", bufs=2) as dgp,
            tc.tile_pool(name="op_", bufs=3) as outp,
            tc.tile_pool(name="stq", bufs=2) as stq,
            tc.tile_pool(name="ps", bufs=3, space="PSUM") as ps,
        ):
            nc.scalar.add_instruction(
                mybir.InstLoadActFuncSet(
                    name=nc.get_next_instruction_name(),
                    ins=[],
                    outs=[],
                    act_func_set_id=act_set_id,
                )
            )
            # Block 0 streams in per-n (first stats start after 256KB, not
            # 2MB); wsb rides between the first two chunks.
            vts = {}
            t0 = vp.tile([128, N, D], f16, tag="v", name="v_0")
            vts[0] = t0
            nc.sync.dma_start(t0[:, 0, :], v[0, 0:128, :])
            wt = wp.tile([128, D], f16, tag="w")
            nc.sync.dma_start(wt[:], wsb[:])
            for n in range(1, N):
                nc.sync.dma_start(t0[:, n, :], v[n, 0:128, :])
            idt = wp.tile([128, 128], f16, tag="id")
            nc.sync.dma_start(idt[:], ident[:])
            epsb = wp.tile([128, 1], f32, tag="eps")
            nc.vector.memset(epsb[:], EPS)
            for pp in range(1, PB):
                t = vp.tile([128, N, D], f16, tag="v", name=f"v_{pp}")
                src = v[:, pp * 128:(pp + 1) * 128, :].rearrange("n p d -> p n d")
                nc.sync.dma_start(t[:], src)
                vts[pp] = t

            stats = {}

            def emit_block_stats(qi, pp):
                lo, hi = QUADS[qi]
                W = (hi - lo) * N
                if qi not in stats:
                    stats[qi] = (
                        stq.tile([128, W], f32, tag=f"ssq{qi}", name=f"ssq_{qi}"),
                        stq.tile([128, W], f32, tag=f"dotq{qi}", name=f"dotq_{qi}"),
                    )
                ssq, dotq = stats[qi]
                if True:
                    j = (pp - lo) * N
                    for n in range(N):
                        dst = dsc.tile([128, D], f16, tag="ds")
                        nc.vector.scalar_tensor_tensor(
                            out=dst[:], in0=vts[pp][:, n, :], scalar=1.0,
                            in1=wt[:], op0=OP.mult, op1=OP.mult,
                            accum_out=dotq[:, j + n:j + n + 1],
                        )
                        if n == 0 and pp in DVE_SQ:
                            dsq = dsc.tile([128, D], f16, tag="ds2")
                            nc.vector.scalar_tensor_tensor(
                                out=dsq[:], in0=vts[pp][:, n, :], scalar=1.0,
                                in1=vts[pp][:, n, :], op0=OP.mult, op1=OP.mult,
                                accum_out=ssq[:, j + n:j + n + 1],
                            )
                        else:
                            sqt = sqc.tile([128, D], f16, tag="sq")
                            nc.scalar.activation(
                                sqt[:], vts[pp][:, n, :], A.Square,
                                accum_out=ssq[:, j + n:j + n + 1],
                            )

            def emit_tail(qi, ssq, dotq):
                lo, hi = QUADS[qi]
                nq = hi - lo
                W = nq * N
                lnq = stq.tile([128, W], f32, tag=f"lnq{qi}", name=f"lnq_{qi}")
                nc.scalar.activation(
                    lnq[:], ssq[:], A.Ln, bias=epsb[:], scale=1.0 / D
                )
                y0q = stq.tile([128, W], f32, tag=f"y0q{qi}", name=f"y0q_{qi}")
                nc.scalar.activation(y0q[:], lnq[:], A.Exp, scale=-0.5)
                lgq = stq.tile([128, W], f32, tag=f"lgq{qi}", name=f"lgq_{qi}")
                nc.vector.tensor_mul(lgq[:], dotq[:], y0q[:])
                nmq = stq.tile([128, nq], f32, tag=f"nmq{qi}", name=f"nmq_{qi}")
                nc.vector.tensor_reduce(
                    nmq[:], lgq[:].rearrange("p (b n) -> p b n", b=nq),
                    X, OP.max, negate=True,
                )
                lgs = stq.tile([128, W], f32, tag=f"lgs{qi}", name=f"lgs_{qi}")
                nm_b = nmq[:].unsqueeze(2).broadcast_to([128, nq, N])
                nc.vector.tensor_tensor(
                    lgs[:].rearrange("p (b n) -> p b n", b=nq),
                    lgq[:].rearrange("p (b n) -> p b n", b=nq),
                    nm_b, OP.add,
                )
                eq = stq.tile([128, W], f16, tag=f"eq{qi}", name=f"eq_{qi}")
                nc.scalar.activation(eq[:], lgs[:], A.Exp)
                smq = stq.tile([128, nq], f32, tag=f"smq{qi}", name=f"smq_{qi}")
                nc.vector.tensor_reduce(
                    smq[:], eq[:].rearrange("p (b n) -> p b n", b=nq),
                    X, OP.add,
                )
                rsq = stq.tile([128, nq], f32, tag=f"rsq{qi}", name=f"rsq_{qi}")
                nc.vector.reciprocal(rsq[:], smq[:])

                dgq = dgp.tile(
                    [128, W * 128], f16, tag="dg", name=f"dg_{qi}"
                )
                e_b = eq[:].unsqueeze(2).broadcast_to([128, W, 128])
                i_b = idt[:].unsqueeze(1).broadcast_to([128, W, 128])
                diag_eng = nc.gpsimd if nq > 1 else nc.vector
                diag_eng.tensor_tensor(
                    dgq[:].rearrange("p (w c) -> p w c", w=W), e_b, i_b, OP.mult
                )

                for pp in range(lo, hi):
                    b = pp - lo
                    acc_ps = ps.tile([128, D], f32, tag="acc")
                    for n in range(N):
                        dgsl = dgq[:, (b * N + n) * 128:(b * N + n + 1) * 128]
                        nc.tensor.matmul(
                            acc_ps[:, 0:512], dgsl, vts[pp][:, n, 0:512],
                            start=(n == 0), stop=(n == N - 1),
                        )
                        nc.tensor.matmul(
                            acc_ps[:, 512:1024], dgsl, vts[pp][:, n, 512:1024],
                            start=(n == 0), stop=(n == N - 1),
                        )
                    outt = outp.tile([128, D], f16, tag="o")
                    rs_b = rsq[:, b:b + 1]
                    if pp in DVE_COPY:
                        nc.vector.tensor_scalar(
                            outt[:], acc_ps[:], rs_b, None, OP.mult
                        )
                    else:
                        nc.scalar.activation(
                            outt[:], acc_ps[:], A.Copy, scale=rs_b
                        )
                    nc.sync.dma_start(o[pp * 128:(pp + 1) * 128, :], outt[:])

            # one-block-lookahead emission: quad qi's tail is emitted after
            # the first block of quad qi+1's stats, so each engine has ~one
            # block of queued stat work while the cross-engine softmax chain
            # of the previous quad resolves.
            stats = {}
            pending = None
            for qi, (lo, hi) in enumerate(QUADS):
                for pp in range(lo, hi):
                    emit_block_stats(qi, pp)
                    if pp == lo and pending is not None:
                        emit_tail(pending, *stats[pending])
                        pending = None
                pending = qi
            emit_tail(pending, *stats[pending])

    nc.compile()
    return nc


def get_program():
    if "nc" not in _cache:
        _cache["nc"] = _build()
    return _cache["nc"]


def make_in_maps(V, proj, scale, block_idx):
    V = np.asarray(V)
    proj = np.asarray(proj, dtype=np.float32)
    scale = np.asarray(scale, dtype=np.float32)
    idx = min(int(block_idx), proj.shape[0] - 1)
    ws = (proj[idx] * scale).astype(np.float16)
    wsb = np.ascontiguousarray(np.broadcast_to(ws, (128, D)))
    eye = np.eye(128, dtype=np.float16)
    Vf = V.reshape(N, BS, D)
    return [
        {
            "v": np.ascontiguousarray(Vf[:, k * PER:(k + 1) * PER, :]).astype(
                np.float16
            ),
            "wsb": wsb,
            "ident": eye,
        }
        for k in range(NCORES)
    ]


def kernel(V, proj, scale, block_idx):
    from concourse.bass_utils import run_bass_kernel_spmd

    nc = get_program()
    in_maps = make_in_maps(V, proj, scale, block_idx)
    res = run_bass_kernel_spmd(nc, in_maps, core_ids=list(range(NCORES)))
    _cache["last_exec_time_ns"] = res.exec_time_ns
    _cache["last_results"] = res
    out = np.concatenate([res.results[k]["o"] for k in range(NCORES)], axis=0)
    return out.reshape(B, S, D).astype(np.float32)
